# revision 29
# baseline (speedup 1.0000x reference)
"""Bitnet-style GQA attention block on 8 trn2 NeuronCores.

Sharding: DP2 (batch) x TP4 (heads). Each core handles one batch element and
8 q-heads / 2 kv-heads, computing its slice of q/k/v proj, attention, and a
partial o-proj (contraction over its 512 attention channels). The host sums
the 4 bf16 partials per batch in f32 and transposes to [S, H].

Device-side layout is feature-major: activations live as [channels, tokens]
so every matmul contracts over the partition dim; all matmuls are bf16 with
fp32 PSUM accumulation. Softmax is unnormalized exp over transposed score
tiles S.T[k, q] (|scores| <= ~5), with the denominator obtained as an extra
all-ones column appended to V in the P@V matmul.

Schedule (v3): x streams as four 512-token stripes [128, 16hk, 512]; K, V
and pair-0 Q projections for stripe 0 run as soon as it lands (~8us) and
attention starts right after, with the remaining stripes' projections
injected into the first block's chunks just ahead of their score/PV
deadlines. The attention loop is qb-outer / head-pair-inner, so each
512-token q-range's A completes after its four pairs and its o-proj is
leveled across the following q-range instead of piling up at the end. For
the last q-range, pairs 0-2 of the o-proj accumulate into bf16 SBUF tiles
during the final block and the kernel tail is only pair 3's 16 single
matmuls + vector adds + output DMAs. The scalar engine runs only exp
([128, 1024] per head-pair chunk); PV trails scores by two chunks; the four
PV q-tile accumulators of a head share one PSUM bank via has_written
first-touch. PSUM: 2x2-bank score tiles, 2 PV banks, 1 held projection
accumulator, 1 rotating transient.
"""

import numpy as np
import ml_dtypes
from contextlib import ExitStack

import concourse.bass as bass
import concourse.tile as tile
from concourse import bacc, mybir
from concourse.bass_utils import run_bass_kernel_spmd
from concourse.masks import make_identity

B, S, H = 2, 2048, 2048
N_HEADS, N_KV, HEAD_DIM = 32, 8, 64
N_CORES = 8
TP = 4                   # head-parallel degree per batch
QH = N_HEADS // TP       # 8 q-heads per core
KVH = N_KV // TP         # 2 kv heads per core
QCH = QH * HEAD_DIM      # 512
KCH = KVH * HEAD_DIM     # 128
ST = S // 128            # 16 token tiles
HK = H // 128            # 16 hidden-dim chunks
NS = 4                   # 512-token x stripes
QB = 4                   # 512-wide q/token column blocks
HEAD_ORDER = [0, 4, 1, 5, 2, 6, 3, 7]  # slot j -> local q-head index

F32 = mybir.dt.float32
BF16 = mybir.dt.bfloat16
BF16_NP = ml_dtypes.bfloat16

_CACHED_NC = None


def _build_nc():
    nc = bacc.Bacc("TRN2", target_bir_lowering=False, debug=False,
                   num_devices=N_CORES)

    # host-prearranged so every input DMA walks fully contiguous memory
    xT5 = nc.dram_tensor("xT5", [NS, 128, HK, 512], BF16,
                         kind="ExternalInput").ap()
    wk4 = nc.dram_tensor("wk4", [128, HK, KCH], BF16, kind="ExternalInput").ap()
    wv4 = nc.dram_tensor("wv4", [128, HK, KCH], BF16, kind="ExternalInput").ap()
    wq5 = nc.dram_tensor("wq5", [4, 128, HK, 128], BF16,
                         kind="ExternalInput").ap()
    woT = nc.dram_tensor("woT", [QCH, H], BF16, kind="ExternalInput").ap()
    outT = nc.dram_tensor("outT", [H, 1536], BF16, kind="ExternalOutput").ap()
    outT3 = nc.dram_tensor("outT3", [H, 512], F32, kind="ExternalOutput").ap()

    with tile.TileContext(nc) as tc, ExitStack() as ctx:
        # ---- pools ----
        xsp = ctx.enter_context(tc.tile_pool(name="xsp", bufs=NS))
        wkp = ctx.enter_context(tc.tile_pool(name="wkp", bufs=2))
        wqp = ctx.enter_context(tc.tile_pool(name="wqp", bufs=4))
        wop = ctx.enter_context(tc.tile_pool(name="wop", bufs=4))
        qtp = ctx.enter_context(tc.tile_pool(name="qtp", bufs=8))
        ktp = ctx.enter_context(tc.tile_pool(name="ktp", bufs=1))
        vp = ctx.enter_context(tc.tile_pool(name="vp", bufs=ST))
        ap_ = ctx.enter_context(tc.tile_pool(name="ap", bufs=4))
        atp = ctx.enter_context(tc.tile_pool(name="atp", bufs=8))
        pexp = ctx.enter_context(tc.tile_pool(name="pexp", bufs=8))
        stg = ctx.enter_context(tc.tile_pool(name="stg", bufs=4))
        rcp = ctx.enter_context(tc.tile_pool(name="rcp", bufs=16))
        oac = ctx.enter_context(tc.tile_pool(name="oac", bufs=ST))
        cst = ctx.enter_context(tc.tile_pool(name="cst", bufs=1))
        # PSUM (8 banks): big = 2 x 2-bank score tiles; pap = 2 x PV
        # accumulators; hld = the single held projection accumulator
        # (pk/pv/pq, one live at a time by schedule); trp = 1 rotating
        # transient (o-proj po / V+A transpose scratch)
        big = ctx.enter_context(tc.tile_pool(name="big", bufs=2, space="PSUM"))
        pap = ctx.enter_context(tc.tile_pool(name="pap", bufs=2, space="PSUM"))
        hld = ctx.enter_context(tc.tile_pool(name="hld", bufs=1, space="PSUM"))
        trp = ctx.enter_context(tc.tile_pool(name="trp", bufs=1, space="PSUM"))

        ident = cst.tile([128, 128], BF16, tag="ident")
        make_identity(nc, ident[:])

        # ---- input DMA. The ring engine is occupied for the duration of
        # each transfer, so the scalar engine (exp) carries nothing and the
        # input stream is split between the sync and gpsimd rings; each x
        # stripe is split into hk halves, one per ring, so stripe 0 lands
        # ~3us after the rings start. ----
        xst = []
        for s_ in range(NS):
            t = xsp.tile([128, HK, 512], BF16, tag="xs", name=f"xs{s_}")
            xst.append(t)
        wk_t = wkp.tile([128, HK, KCH], BF16, tag="wk")
        wv_t = wkp.tile([128, HK, KCH], BF16, tag="wk", name="wv")
        wq_t = [wqp.tile([128, HK, 128], BF16, tag="wq", name=f"wq{p}")
                for p in range(4)]
        nc.sync.dma_start(wk_t[:], wk4[:, :, :])
        nc.gpsimd.dma_start(xst[0][:, 4:8, :], xT5[0, :, 4:8, :])
        nc.sync.dma_start(xst[0][:, 0:4, :], xT5[0, :, 0:4, :])
        nc.gpsimd.dma_start(xst[0][:, 12:HK, :], xT5[0, :, 12:HK, :])
        nc.sync.dma_start(xst[0][:, 8:12, :], xT5[0, :, 8:12, :])
        nc.gpsimd.dma_start(wv_t[:], wv4[:, :, :])
        nc.sync.dma_start(wq_t[0][:], wq5[0, :, :, :])
        for s_ in range(1, NS):
            nc.sync.dma_start(xst[s_][:, 0:8, :], xT5[s_, :, 0:8, :])
            nc.gpsimd.dma_start(xst[s_][:, 8:HK, :], xT5[s_, :, 8:HK, :])
            nc.gpsimd.dma_start(wq_t[s_][:], wq5[s_, :, :, :])
        wo = []

        # ---- PE warm-up: junk transposes while the first stripe streams,
        # so the tensor engine is at full p-state when K(0) starts ----
        wrm = big.tile([128, 1024], BF16, tag="big", name="warm")
        for _ in range(32):
            nc.tensor.transpose(wrm[:, 0:128], ident[:], ident[:])

        # wo rides the tail of the gpsimd input queue (after the x stream)
        for i in range(4):
            t = wop.tile([128, H], BF16, tag="wo", name=f"wo{i}")
            nc.gpsimd.dma_start(t[:], woT[i * 128:(i + 1) * 128, :])
            wo.append(t)

        # ---- K projection, stripe-wise halves ----
        kt_sb = ktp.tile([128, S], BF16, tag="kt")
        pk_of = {}

        def emit_k(s, half):
            if half == 0:
                pk_of[s] = hld.tile([128, 512], F32, tag="hl", name=f"pk{s}")
            pk = pk_of[s]
            for hk in range(half * 8, half * 8 + 8):
                nc.tensor.matmul(pk[:], wk_t[:, hk:hk + 1, :],
                                 xst[s][:, hk:hk + 1, :],
                                 start=(hk == 0), stop=(hk == HK - 1))
            if half == 1:
                nc.vector.tensor_copy(kt_sb[:, s * 512:(s + 1) * 512], pk[:])

        # ---- V projection, stripe-wise; V.T[ch, tok] -> token-major
        # Vones[tok, 130] (V | 1 interleaved per kv head) ----
        vones = [vp.tile([128, 130], BF16, tag="vones", name=f"vt{st}")
                 for st in range(ST)]
        for st in range(ST):
            nc.vector.memset(vones[st][:, 64:65], 1.0)
            nc.vector.memset(vones[st][:, 129:130], 1.0)

        pv_of = {}

        def emit_v(s, half):
            if half == 0:
                pv_of[s] = (hld.tile([128, 512], F32, tag="hl", name=f"pv{s}"),
                            stg.tile([128, 512], BF16, tag="stg",
                                     name=f"vts{s}"))
            pvt, _ = pv_of[s]
            for hk in range(half * 8, half * 8 + 8):
                nc.tensor.matmul(pvt[:], wv_t[:, hk:hk + 1, :],
                                 xst[s][:, hk:hk + 1, :],
                                 start=(hk == 0), stop=(hk == HK - 1))
            if half == 1:
                nc.vector.tensor_copy(pv_of[s][1][:], pvt[:])

        def emit_vt(s, jj):
            # transpose two 128-token chunks of V.T via one psum scratch
            vtsb = pv_of[s][1]
            pt = trp.tile([128, 256], BF16, tag="tr", name="ptv")
            for i, j in enumerate(jj):
                nc.tensor.transpose(pt[:, i * 128:(i + 1) * 128],
                                    vtsb[:, j * 128:(j + 1) * 128], ident[:])
            for i, j in enumerate(jj):
                st = s * 4 + j
                nc.vector.tensor_copy(vones[st][:, 0:64],
                                      pt[:, i * 128:i * 128 + 64])
                nc.vector.tensor_copy(vones[st][:, 65:129],
                                      pt[:, i * 128 + 64:(i + 1) * 128])

        # ---- Q projection: per (pair, stripe) in 4-hk quarters ----
        qpad_of = {}

        def emit_qpad_alloc(t):
            qpad = []
            for h in range(2):
                qp = qtp.tile([128, S], BF16, tag="qt", name=f"qp{t}_{h}")
                lo = (1 - h) * 64  # zero half
                nc.vector.memset(qp[lo:lo + 64, :], 0.0)
                qpad.append(qp)
            qpad_of[t] = qpad

        pq_of = {}

        def emit_q(p, s, quarter, npieces=4):
            if quarter == 0:
                pq_of[(p, s)] = hld.tile([128, 512], F32, tag="hl",
                                         name=f"pq{p}_{s}")
            pq = pq_of[(p, s)]
            step = HK // npieces
            for hk in range(quarter * step, (quarter + 1) * step):
                nc.tensor.matmul(pq[:], wq_t[p][:, hk:hk + 1, :],
                                 xst[s][:, hk:hk + 1, :],
                                 start=(hk == 0), stop=(hk == HK - 1))
            if quarter == npieces - 1:
                qpad = qpad_of[p]
                cols = slice(s * 512, (s + 1) * 512)
                nc.vector.tensor_copy(qpad[0][0:64, cols], pq[0:64, :])
                nc.vector.tensor_copy(qpad[1][64:128, cols], pq[64:128, :])

        # ---- deferred A-transpose: pair t's 128 A-channels of block (qb,t)
        # transpose into the o-proj operand (ak == t) early in the NEXT
        # block, by which time the DVE normalize has drained ----
        a_of = {}
        at_of = {qb: [] for qb in range(QB)}

        def emit_at(qb, t):
            at_t = atp.tile([128, 512], BF16, tag="at", name=f"at{qb}_{t}")
            ptb = trp.tile([128, 512], BF16, tag="tr", name="ptb")
            for sq in range(4):
                nc.tensor.transpose(
                    ptb[:, sq * 128:(sq + 1) * 128],
                    a_of[qb][sq][:, t * 128:(t + 1) * 128], ident[:])
            nc.vector.tensor_copy(at_t[:], ptb[:])
            at_of[qb].append(at_t)
        o_acc = [oac.tile([128, 512], BF16, tag="oa", name=f"oa{ot}")
                 for ot in range(ST)]

        def emit_oproj_ot(qb, ot):
            # full o-proj column tile for qb (qb 0..2): 4 ak matmuls
            po = trp.tile([128, 512], F32, tag="tr", name="po")
            for ak in range(4):
                nc.tensor.matmul(po[:], wo[ak][:, ot * 128:(ot + 1) * 128],
                                 at_of[qb][ak][:],
                                 start=(ak == 0), stop=(ak == 3))
            so = stg.tile([128, 512], BF16, tag="stg")
            nc.vector.tensor_copy(so[:], po[:])
            nc.sync.dma_start(
                outT[ot * 128:(ot + 1) * 128, qb * 512:(qb + 1) * 512], so[:])

        def emit_oproj3_acc(ot):
            # last q-range, pairs 0-2 -> bf16 SBUF accumulator
            po = trp.tile([128, 512], F32, tag="tr", name="po3")
            for ak in range(3):
                nc.tensor.matmul(po[:], wo[ak][:, ot * 128:(ot + 1) * 128],
                                 at_of[QB - 1][ak][:],
                                 start=(ak == 0), stop=(ak == 2))
            nc.vector.tensor_copy(o_acc[ot][:], po[:])

        def emit_oproj3_tail(ot):
            # pair 3's contribution; the pairs-0-2 accumulator is folded in
            # with an identity matmul and the result DMAs straight from psum
            # (alternate psum pools and DMA rings so consecutive tiles
            # pipeline)
            pool = trp if ot % 2 == 0 else hld
            po = pool.tile([128, 512], F32, tag="tr" if ot % 2 == 0 else "hl",
                           name="pot")
            nc.tensor.matmul(po[:], wo[3][:, ot * 128:(ot + 1) * 128],
                             at_of[QB - 1][3][:], start=True, stop=False)
            nc.tensor.matmul(po[:], ident[:], o_acc[ot][:],
                             start=False, stop=True)
            so = stg.tile([128, 512], F32, tag="stg", name="so3")
            if ot % 2 == 0:
                nc.vector.tensor_copy(so[:], po[:])
            else:
                nc.scalar.activation(so[:], po[:],
                                     mybir.ActivationFunctionType.Copy)
            ring = nc.sync if ot % 2 == 0 else nc.gpsimd
            ring.dma_start(outT3[ot * 128:(ot + 1) * 128, :], so[:])

        # ---- static injection schedule ----
        def KH(s, h):
            return lambda: emit_k(s, h)

        def VH(s, h):
            return lambda: emit_v(s, h)

        def VT(s, jj):
            return lambda: emit_vt(s, jj)

        def QQ(p, s, q):
            return lambda: emit_q(p, s, q)

        def QH2(p, s, h):
            return lambda: emit_q(p, s, h, npieces=2)

        def QA(t):
            return lambda: emit_qpad_alloc(t)

        def OO(qb, ot):
            return lambda: emit_oproj_ot(qb, ot)

        def O3(ot):
            return lambda: emit_oproj3_acc(ot)

        def AT(qb, t):
            return lambda: emit_at(qb, t)

        sched = {}

        def put(qb, t, kt, *fns):
            sched.setdefault((qb, t, kt), []).extend(fns)

        # (0,0): V(0) plus the remaining K/V stripes, each just ahead of its
        # score/PV deadline, chained one-at-a-time through the hld bank;
        # Q(1,0) completes within the block (its qpad is read at (0,1,0))
        put(0, 0, 0, VH(0, 0))
        put(0, 0, 1, VH(0, 1), VT(0, (0, 1)))
        put(0, 0, 2, VT(0, (2, 3)), KH(1, 0))
        put(0, 0, 3, KH(1, 1))
        put(0, 0, 4, VH(1, 0))
        put(0, 0, 5, VH(1, 1), VT(1, (0, 1)))
        put(0, 0, 6, VT(1, (2, 3)), KH(2, 0))
        put(0, 0, 7, KH(2, 1))
        put(0, 0, 8, VH(2, 0))
        put(0, 0, 9, VH(2, 1), VT(2, (0, 1)))
        put(0, 0, 10, VT(2, (2, 3)), KH(3, 0))
        put(0, 0, 11, KH(3, 1), QA(1))
        put(0, 0, 12, VH(3, 0))
        put(0, 0, 13, VH(3, 1), VT(3, (0, 1)))
        put(0, 0, 14, VT(3, (2, 3)), QH2(1, 0, 0))
        put(0, 0, 15, QH2(1, 0, 1))
        put(0, 1, 0, QA(2))
        for q in range(4):
            put(0, 1, 1 + 2 * q, QQ(2, 0, q))
        put(0, 1, 9, QA(3))
        for q in range(4):
            put(0, 1, 10 + q, QQ(3, 0, q))
        # steady-state Q-pair pipeline: Q(p, s) ready before block (qb=s, t=p)
        qseq = [(0, 1), (1, 1), (2, 1), (3, 1), (0, 2), (1, 2), (2, 2),
                (3, 2), (0, 3), (1, 3), (2, 3), (3, 3)]
        blocks = [(0, 2), (0, 3), (1, 0), (1, 1), (1, 2), (1, 3), (2, 0),
                  (2, 1), (2, 2), (2, 3), (3, 0), (3, 1)]
        for (p, s_), (bqb, bt) in zip(qseq, blocks):
            for q in range(4):
                put(bqb, bt, 2 * q, QQ(p, s_, q))
        # deferred A-transposes: early in the block after the pair completes
        atseq = [(qb, t) for qb in range(QB) for t in range(4)][:-1]
        atblk = [(qb, t) for qb in range(QB) for t in range(4)][1:]
        for (aqb, at_), (bqb, bt) in zip(atseq, atblk):
            put(bqb, bt, 2 if (bqb, bt) == (0, 1) else 1, AT(aqb, at_))
        # o-proj for qb leveled across the next q-range (qb 0..2 -> qb+1;
        # qb 2's last tiles land in the first three blocks of qb 3)
        for qb in range(2):
            for ot in range(ST):
                put(qb + 1, ot // 4, 3 + 4 * (ot % 4), OO(qb, ot))
        oo2 = [(0, 3), (0, 5), (0, 7), (0, 9), (0, 11), (0, 13),
               (1, 3), (1, 5), (1, 7), (1, 9), (1, 11), (1, 13),
               (2, 3), (2, 5), (2, 7), (2, 9)]
        for ot, (bt, kt_) in enumerate(oo2):
            put(3, bt, kt_, OO(2, ot))
        # last q-range: pairs 0-2 of its o-proj during the final block
        o3kt = [2, 2, 3, 3, 4, 5, 6, 7, 8, 9, 10, 11, 12, 13, 14, 15]
        for ot in range(ST):
            put(3, 3, o3kt[ot], O3(ot))

        # ---- pre-attention: stripe-0 K and pair-0 Q (V(0) rides the first
        # two chunks) ----
        emit_qpad_alloc(0)
        emit_k(0, 0)
        emit_k(0, 1)
        emit_q(0, 0, 0)
        emit_q(0, 0, 1)
        emit_q(0, 0, 2)
        emit_q(0, 0, 3)

        # ---- attention: qb outer, head-pair inner ----
        for qb in range(QB):
            qcols = slice(qb * 512, (qb + 1) * 512)
            a_tiles = [ap_.tile([128, QCH], BF16, tag="a", name=f"a{qb}_{i}")
                       for i in range(4)]
            a_of[qb] = a_tiles
            for t in range(4):
                qpad = qpad_of[t]
                ptile = [None] * ST
                pa = [pap.tile([128, 260], F32, tag="pa", name=f"pa{h}")
                      for h in range(2)]

                def emit_pv(kt):
                    for h in range(2):
                        for qt in range(4):
                            nc.tensor.matmul(
                                pa[h][:, qt * 65:qt * 65 + 65],
                                ptile[kt][:, h * 512 + qt * 128:
                                          h * 512 + (qt + 1) * 128],
                                vones[kt][:, h * 65:h * 65 + 65],
                                start=(kt == 0 and qt == 0),
                                stop=(kt == ST - 1 and qt == 3),
                                skip_group_check=True)

                for kt in range(ST):
                    ps2 = big.tile([128, 1024], F32, tag="big")
                    for h in range(2):
                        nc.tensor.matmul(
                            ps2[:, h * 512:(h + 1) * 512],
                            kt_sb[:, kt * 128:(kt + 1) * 128],
                            qpad[h][:, qcols],
                            start=True, stop=True)
                    pe = pexp.tile([128, 1024], BF16, tag="pexp")
                    nc.scalar.activation(pe[:], ps2[:],
                                         mybir.ActivationFunctionType.Exp,
                                         scale=0.125)
                    ptile[kt] = pe
                    if kt >= 2:
                        emit_pv(kt - 2)
                    for f in sched.get((qb, t, kt), []):
                        f()
                emit_pv(ST - 2)
                emit_pv(ST - 1)

                for h in range(2):
                    slot = 2 * t + h
                    for qt in range(4):
                        rc = rcp.tile([128, 1], F32, tag="rc")
                        nc.vector.reciprocal(
                            rc[:], pa[h][:, qt * 65 + 64:qt * 65 + 65])
                        nc.vector.tensor_scalar_mul(
                            a_tiles[qt][:, slot * 64:(slot + 1) * 64],
                            pa[h][:, qt * 65:qt * 65 + 64], rc[:])

        # tail: pair 3 of the last q-range
        emit_at(QB - 1, 3)
        for ot in range(ST):
            emit_oproj3_tail(ot)

    nc.compile()
    return nc


def _get_nc():
    global _CACHED_NC
    if _CACHED_NC is None:
        _CACHED_NC = _build_nc()
    return _CACHED_NC


def _arr4(a2d):
    """[H, n] -> [128, HK, n] hk-stripe layout, bf16 contiguous."""
    n = a2d.shape[1]
    return np.ascontiguousarray(
        a2d.reshape(HK, 128, n).transpose(1, 0, 2)).astype(BF16_NP)


def _arr5(xT):
    """[H, S] -> [NS, 128, HK, 512] stripe-major layout, bf16 contiguous."""
    return np.ascontiguousarray(
        xT.reshape(HK, 128, NS, 512).transpose(2, 1, 0, 3)).astype(BF16_NP)


def _prep_core_inputs(hidden_states, Wq, Wk, Wv, Wo):
    """Host-side shard + transpose + bf16 cast. Returns list of 8 input dicts."""
    xT_b = []
    for b in range(B):
        xT_b.append(_arr5(np.ascontiguousarray(hidden_states[b].T)))
    in_maps = []
    for c in range(N_CORES):
        b, g = divmod(c, TP)
        wq_rows = np.concatenate([
            Wq[(g * QH + h) * HEAD_DIM:(g * QH + h + 1) * HEAD_DIM, :]
            for h in HEAD_ORDER], axis=0)            # [512, H]
        wo_cols = np.concatenate([
            Wo[:, (g * QH + h) * HEAD_DIM:(g * QH + h + 1) * HEAD_DIM]
            for h in HEAD_ORDER], axis=1)            # [H, 512]
        wq4 = _arr4(np.ascontiguousarray(wq_rows.T))  # [128, HK, 512]
        wq5 = np.ascontiguousarray(wq4.reshape(128, HK, 4, 128)
                                   .transpose(2, 0, 1, 3))
        in_maps.append({
            "xT5": xT_b[b],
            "wq5": wq5,
            "wk4": _arr4(np.ascontiguousarray(Wk[g * KCH:(g + 1) * KCH, :].T)),
            "wv4": _arr4(np.ascontiguousarray(Wv[g * KCH:(g + 1) * KCH, :].T)),
            "woT": np.ascontiguousarray(wo_cols.T).astype(BF16_NP),
        })
    return in_maps


def _combine(results):
    out = np.empty((B, S, H), dtype=np.float32)
    for b in range(B):
        acc = np.concatenate(
            [results[b * TP]["outT"].astype(np.float32),
             results[b * TP]["outT3"]], axis=1)
        for g in range(1, TP):
            r = results[b * TP + g]
            acc[:, 0:1536] += r["outT"].astype(np.float32)
            acc[:, 1536:2048] += r["outT3"]
        out[b] = acc.T
    return out


def kernel(hidden_states, attention_mask, Wq, Wk, Wv, Wo):
    # attention_mask is all zeros for this problem spec; softmax is invariant
    # to the zero additive mask, so it is not shipped to the device.
    hidden_states = np.asarray(hidden_states)
    nc = _get_nc()
    in_maps = _prep_core_inputs(hidden_states, np.asarray(Wq), np.asarray(Wk),
                                np.asarray(Wv), np.asarray(Wo))
    res = run_bass_kernel_spmd(nc, in_maps, list(range(N_CORES)))
    return _combine(res.results)


# revision 37
# speedup vs baseline: 1.0302x; 1.0302x over previous
"""Bitnet-style GQA attention block on 8 trn2 NeuronCores.

Sharding: DP2 (batch) x TP4 (heads). Each core handles one batch element and
8 q-heads / 2 kv-heads, computing its slice of q/k/v proj, attention, and a
partial o-proj (contraction over its 512 attention channels). The host sums
the 4 bf16 partials per batch in f32 and transposes to [S, H].

Device-side layout is feature-major: activations live as [channels, tokens]
so every matmul contracts over the partition dim; all matmuls are bf16 with
fp32 PSUM accumulation. Softmax is unnormalized exp over transposed score
tiles S.T[k, q] (|scores| <= ~5), with the denominator obtained as an extra
all-ones column appended to V in the P@V matmul.

Schedule (v3): x streams as four 512-token stripes [128, 16hk, 512]; K, V
and pair-0 Q projections for stripe 0 run as soon as it lands (~8us) and
attention starts right after, with the remaining stripes' projections
injected into the first block's chunks just ahead of their score/PV
deadlines. The attention loop is qb-outer / head-pair-inner, so each
512-token q-range's A completes after its four pairs and its o-proj is
leveled across the following q-range instead of piling up at the end. For
the last q-range, pairs 0-2 of the o-proj accumulate into bf16 SBUF tiles
during the final block and the kernel tail is only pair 3's 16 single
matmuls + vector adds + output DMAs. The scalar engine runs only exp
([128, 1024] per head-pair chunk); PV trails scores by two chunks; the four
PV q-tile accumulators of a head share one PSUM bank via has_written
first-touch. PSUM: 2x2-bank score tiles, 2 PV banks, 1 held projection
accumulator, 1 rotating transient.
"""

import numpy as np
import ml_dtypes
from contextlib import ExitStack

import concourse.bass as bass
import concourse.tile as tile
from concourse import bacc, mybir
from concourse.bass_utils import run_bass_kernel_spmd
from concourse.masks import make_identity

B, S, H = 2, 2048, 2048
N_HEADS, N_KV, HEAD_DIM = 32, 8, 64
N_CORES = 8
TP = 4                   # head-parallel degree per batch
QH = N_HEADS // TP       # 8 q-heads per core
KVH = N_KV // TP         # 2 kv heads per core
QCH = QH * HEAD_DIM      # 512
KCH = KVH * HEAD_DIM     # 128
ST = S // 128            # 16 token tiles
HK = H // 128            # 16 hidden-dim chunks
NS = 4                   # 512-token x stripes
QB = 4                   # 512-wide q/token column blocks
HEAD_ORDER = [0, 4, 1, 5, 2, 6, 3, 7]  # slot j -> local q-head index

F32 = mybir.dt.float32
BF16 = mybir.dt.bfloat16
BF16_NP = ml_dtypes.bfloat16

_CACHED_NC = None


def _build_nc():
    nc = bacc.Bacc("TRN2", target_bir_lowering=False, debug=False,
                   num_devices=N_CORES)

    # host-prearranged so every input DMA walks fully contiguous memory
    xT5 = nc.dram_tensor("xT5", [NS, 128, HK, 512], BF16,
                         kind="ExternalInput").ap()
    wk4 = nc.dram_tensor("wk4", [128, HK, KCH], BF16, kind="ExternalInput").ap()
    wv4 = nc.dram_tensor("wv4", [128, HK, KCH], BF16, kind="ExternalInput").ap()
    wq5 = nc.dram_tensor("wq5", [4, 128, HK, 128], BF16,
                         kind="ExternalInput").ap()
    woT = nc.dram_tensor("woT", [QCH, H], BF16, kind="ExternalInput").ap()
    outT = nc.dram_tensor("outT", [H, 1536], BF16, kind="ExternalOutput").ap()
    outT3 = nc.dram_tensor("outT3", [H, 512], F32, kind="ExternalOutput").ap()

    with tile.TileContext(nc) as tc, ExitStack() as ctx:
        # ---- pools ----
        xsp = ctx.enter_context(tc.tile_pool(name="xsp", bufs=NS))
        wkp = ctx.enter_context(tc.tile_pool(name="wkp", bufs=2))
        wqp = ctx.enter_context(tc.tile_pool(name="wqp", bufs=4))
        wop = ctx.enter_context(tc.tile_pool(name="wop", bufs=4))
        qtp = ctx.enter_context(tc.tile_pool(name="qtp", bufs=8))
        ktp = ctx.enter_context(tc.tile_pool(name="ktp", bufs=1))
        vp = ctx.enter_context(tc.tile_pool(name="vp", bufs=ST))
        ap_ = ctx.enter_context(tc.tile_pool(name="ap", bufs=8))
        atp = ctx.enter_context(tc.tile_pool(name="atp", bufs=8))
        pexp = ctx.enter_context(tc.tile_pool(name="pexp", bufs=8))
        stg = ctx.enter_context(tc.tile_pool(name="stg", bufs=4))
        rcp = ctx.enter_context(tc.tile_pool(name="rcp", bufs=16))
        oac = ctx.enter_context(tc.tile_pool(name="oac", bufs=ST))
        cst = ctx.enter_context(tc.tile_pool(name="cst", bufs=1))
        # PSUM (8 banks): big = 2 x 2-bank score tiles; pap = 2 x PV
        # accumulators; hld = the single held projection accumulator
        # (pk/pv/pq, one live at a time by schedule); trp = 1 rotating
        # transient (o-proj po / V+A transpose scratch)
        big = ctx.enter_context(tc.tile_pool(name="big", bufs=2, space="PSUM"))
        pap = ctx.enter_context(tc.tile_pool(name="pap", bufs=2, space="PSUM"))
        hld = ctx.enter_context(tc.tile_pool(name="hld", bufs=1, space="PSUM"))
        trp = ctx.enter_context(tc.tile_pool(name="trp", bufs=1, space="PSUM"))

        ident = cst.tile([128, 128], BF16, tag="ident")
        make_identity(nc, ident[:])

        # ---- input DMA. The ring engine is occupied for the duration of
        # each transfer, so the scalar engine (exp) carries nothing and the
        # input stream is split between the sync and gpsimd rings; each x
        # stripe is split into hk halves, one per ring, so stripe 0 lands
        # ~3us after the rings start. ----
        xst = []
        for s_ in range(NS):
            t = xsp.tile([128, HK, 512], BF16, tag="xs", name=f"xs{s_}")
            xst.append(t)
        wk_t = wkp.tile([128, HK, KCH], BF16, tag="wk")
        wv_t = wkp.tile([128, HK, KCH], BF16, tag="wk", name="wv")
        wq_t = [wqp.tile([128, HK, 128], BF16, tag="wq", name=f"wq{p}")
                for p in range(4)]
        nc.sync.dma_start(wk_t[:], wk4[:, :, :])
        nc.gpsimd.dma_start(xst[0][:, 4:8, :], xT5[0, :, 4:8, :])
        nc.sync.dma_start(xst[0][:, 0:4, :], xT5[0, :, 0:4, :])
        nc.gpsimd.dma_start(xst[0][:, 12:HK, :], xT5[0, :, 12:HK, :])
        nc.sync.dma_start(xst[0][:, 8:12, :], xT5[0, :, 8:12, :])
        nc.gpsimd.dma_start(wv_t[:], wv4[:, :, :])
        nc.sync.dma_start(wq_t[0][:], wq5[0, :, :, :])
        for s_ in range(1, NS):
            nc.sync.dma_start(xst[s_][:, 0:8, :], xT5[s_, :, 0:8, :])
            nc.gpsimd.dma_start(xst[s_][:, 8:HK, :], xT5[s_, :, 8:HK, :])
            nc.gpsimd.dma_start(wq_t[s_][:], wq5[s_, :, :, :])
        wo = []

        # ---- PE warm-up: junk transposes while the first stripe streams,
        # so the tensor engine is at full p-state when K(0) starts ----
        wrm = big.tile([128, 1024], BF16, tag="big", name="warm")
        for _ in range(32):
            nc.tensor.transpose(wrm[:, 0:128], ident[:], ident[:])

        # wo rides the tail of the gpsimd input queue (after the x stream)
        for i in range(4):
            t = wop.tile([128, H], BF16, tag="wo", name=f"wo{i}")
            nc.gpsimd.dma_start(t[:], woT[i * 128:(i + 1) * 128, :])
            wo.append(t)

        # ---- K projection, stripe-wise halves ----
        kt_sb = ktp.tile([128, S], BF16, tag="kt")
        pk_of = {}

        def emit_k(s, half):
            if half == 0:
                pk_of[s] = hld.tile([128, 512], F32, tag="hl", name=f"pk{s}")
            pk = pk_of[s]
            for hk in range(half * 8, half * 8 + 8):
                nc.tensor.matmul(pk[:], wk_t[:, hk:hk + 1, :],
                                 xst[s][:, hk:hk + 1, :],
                                 start=(hk == 0), stop=(hk == HK - 1))
            if half == 1:
                nc.vector.tensor_copy(kt_sb[:, s * 512:(s + 1) * 512], pk[:])

        # ---- V projection, stripe-wise; V.T[ch, tok] -> token-major
        # Vones[tok, 130] (V | 1 interleaved per kv head) ----
        vones = [vp.tile([128, 130], BF16, tag="vones", name=f"vt{st}")
                 for st in range(ST)]
        for st in range(ST):
            nc.vector.memset(vones[st][:, 64:65], 1.0)
            nc.vector.memset(vones[st][:, 129:130], 1.0)

        pv_of = {}

        def emit_v(s, half):
            if half == 0:
                pv_of[s] = (hld.tile([128, 512], F32, tag="hl", name=f"pv{s}"),
                            stg.tile([128, 512], BF16, tag="stg",
                                     name=f"vts{s}"))
            pvt, _ = pv_of[s]
            for hk in range(half * 8, half * 8 + 8):
                nc.tensor.matmul(pvt[:], wv_t[:, hk:hk + 1, :],
                                 xst[s][:, hk:hk + 1, :],
                                 start=(hk == 0), stop=(hk == HK - 1))
            if half == 1:
                nc.vector.tensor_copy(pv_of[s][1][:], pvt[:])

        def emit_vt(s, jj):
            # transpose two 128-token chunks of V.T via one psum scratch
            vtsb = pv_of[s][1]
            pt = trp.tile([128, 256], BF16, tag="tr", name="ptv")
            for i, j in enumerate(jj):
                nc.tensor.transpose(pt[:, i * 128:(i + 1) * 128],
                                    vtsb[:, j * 128:(j + 1) * 128], ident[:])
            for i, j in enumerate(jj):
                st = s * 4 + j
                nc.vector.tensor_copy(vones[st][:, 0:64],
                                      pt[:, i * 128:i * 128 + 64])
                nc.vector.tensor_copy(vones[st][:, 65:129],
                                      pt[:, i * 128 + 64:(i + 1) * 128])

        # ---- Q projection: per (pair, stripe) in 4-hk quarters ----
        qpad_of = {}

        def emit_qpad_alloc(t):
            qpad = []
            for h in range(2):
                qp = qtp.tile([128, S], BF16, tag="qt", name=f"qp{t}_{h}")
                lo = (1 - h) * 64  # zero half
                nc.vector.memset(qp[lo:lo + 64, :], 0.0)
                qpad.append(qp)
            qpad_of[t] = qpad

        pq_of = {}

        def emit_q(p, s, quarter, npieces=4):
            if quarter == 0:
                pq_of[(p, s)] = hld.tile([128, 512], F32, tag="hl",
                                         name=f"pq{p}_{s}")
            pq = pq_of[(p, s)]
            step = HK // npieces
            for hk in range(quarter * step, (quarter + 1) * step):
                nc.tensor.matmul(pq[:], wq_t[p][:, hk:hk + 1, :],
                                 xst[s][:, hk:hk + 1, :],
                                 start=(hk == 0), stop=(hk == HK - 1))
            if quarter == npieces - 1:
                qpad = qpad_of[p]
                cols = slice(s * 512, (s + 1) * 512)
                nc.vector.tensor_copy(qpad[0][0:64, cols], pq[0:64, :])
                nc.vector.tensor_copy(qpad[1][64:128, cols], pq[64:128, :])

        # ---- deferred A-transpose: pair t's 128 A-channels of block (qb,t)
        # transpose into the o-proj operand (ak == t) early in the NEXT
        # block, by which time the DVE normalize has drained ----
        a_of = {}
        at_of = {qb: [] for qb in range(QB)}

        def emit_at(qb, t):
            at_t = atp.tile([128, 512], BF16, tag="at", name=f"at{qb}_{t}")
            ptb = trp.tile([128, 512], BF16, tag="tr", name="ptb")
            for sq in range(4):
                nc.tensor.transpose(
                    ptb[:, sq * 128:(sq + 1) * 128],
                    a_of[qb][sq][:, t * 128:(t + 1) * 128], ident[:])
            nc.vector.tensor_copy(at_t[:], ptb[:])
            at_of[qb].append(at_t)
        o_acc = [oac.tile([128, 512], BF16, tag="oa", name=f"oa{ot}")
                 for ot in range(ST)]

        def emit_oproj_ot(qb, ot):
            # full o-proj column tile for qb (qb 0..2): 4 ak matmuls
            po = trp.tile([128, 512], F32, tag="tr", name="po")
            for ak in range(4):
                nc.tensor.matmul(po[:], wo[ak][:, ot * 128:(ot + 1) * 128],
                                 at_of[qb][ak][:],
                                 start=(ak == 0), stop=(ak == 3))
            so = stg.tile([128, 512], BF16, tag="stg")
            nc.vector.tensor_copy(so[:], po[:])
            nc.sync.dma_start(
                outT[ot * 128:(ot + 1) * 128, qb * 512:(qb + 1) * 512], so[:])

        def emit_oproj3_acc(ot):
            # last q-range, pairs 0-2 -> bf16 SBUF accumulator (alternate
            # psum pools so consecutive units pipeline; hld is idle here)
            pool, tg = (trp, "tr") if ot % 2 == 0 else (hld, "hl")
            po = pool.tile([128, 512], F32, tag=tg, name="po3")
            for ak in range(3):
                nc.tensor.matmul(po[:], wo[ak][:, ot * 128:(ot + 1) * 128],
                                 at_of[QB - 1][ak][:],
                                 start=(ak == 0), stop=(ak == 2))
            nc.vector.tensor_copy(o_acc[ot][:], po[:])

        def emit_oproj3_tail(ot):
            # pair 3's contribution; the pairs-0-2 accumulator is folded in
            # with an identity matmul and the result DMAs straight from psum
            # (alternate psum pools and DMA rings so consecutive tiles
            # pipeline)
            pool = trp if ot % 2 == 0 else hld
            po = pool.tile([128, 512], F32, tag="tr" if ot % 2 == 0 else "hl",
                           name="pot")
            nc.tensor.matmul(po[:], wo[3][:, ot * 128:(ot + 1) * 128],
                             at_of[QB - 1][3][:], start=True, stop=False)
            nc.tensor.matmul(po[:], ident[:], o_acc[ot][:],
                             start=False, stop=True)
            so = stg.tile([128, 512], F32, tag="stg", name="so3")
            if ot % 2 == 0:
                nc.vector.tensor_copy(so[:], po[:])
            else:
                nc.scalar.activation(so[:], po[:],
                                     mybir.ActivationFunctionType.Copy)
            ring = nc.sync if ot % 2 == 0 else nc.gpsimd
            ring.dma_start(outT3[ot * 128:(ot + 1) * 128, :], so[:])

        # ---- static injection schedule ----
        def KH(s, h):
            return lambda: emit_k(s, h)

        def VH(s, h):
            return lambda: emit_v(s, h)

        def VT(s, jj):
            return lambda: emit_vt(s, jj)

        def QQ(p, s, q):
            return lambda: emit_q(p, s, q)

        def QH2(p, s, h):
            return lambda: emit_q(p, s, h, npieces=2)

        def QA(t):
            return lambda: emit_qpad_alloc(t)

        def OO(qb, ot):
            return lambda: emit_oproj_ot(qb, ot)

        def O3(ot):
            return lambda: emit_oproj3_acc(ot)

        def AT(qb, t):
            return lambda: emit_at(qb, t)

        sched = {}

        def put(qb, t, kt, *fns):
            sched.setdefault((qb, t, kt), []).extend(fns)

        # (0,0): V(0) plus the remaining K/V stripes, each just ahead of its
        # score/PV deadline, chained one-at-a-time through the hld bank;
        # Q(1,0) completes within the block (its qpad is read at (0,1,0))
        put(0, 0, 0, VH(0, 0))
        put(0, 0, 1, VH(0, 1), VT(0, (0, 1)))
        put(0, 0, 2, VT(0, (2, 3)), KH(1, 0))
        put(0, 0, 3, KH(1, 1))
        put(0, 0, 4, VH(1, 0))
        put(0, 0, 5, VH(1, 1), VT(1, (0, 1)))
        put(0, 0, 6, VT(1, (2, 3)), KH(2, 0))
        put(0, 0, 7, KH(2, 1))
        put(0, 0, 8, VH(2, 0))
        put(0, 0, 9, VH(2, 1), VT(2, (0, 1)))
        put(0, 0, 10, VT(2, (2, 3)), KH(3, 0))
        put(0, 0, 11, KH(3, 1), QA(1))
        put(0, 0, 12, VH(3, 0))
        put(0, 0, 13, VH(3, 1), VT(3, (0, 1)))
        put(0, 0, 14, VT(3, (2, 3)), QH2(1, 0, 0))
        put(0, 0, 15, QH2(1, 0, 1))
        put(0, 1, 0, QA(2))
        for q in range(4):
            put(0, 1, 1 + 2 * q, QQ(2, 0, q))
        put(0, 1, 9, QA(3))
        for q in range(4):
            put(0, 1, 10 + q, QQ(3, 0, q))
        # steady-state Q-pair pipeline: Q(p, s) ready before block (qb=s, t=p)
        qseq = [(0, 1), (1, 1), (2, 1), (3, 1), (0, 2), (1, 2), (2, 2),
                (3, 2), (0, 3), (1, 3), (2, 3), (3, 3)]
        blocks = [(0, 2), (0, 3), (1, 0), (1, 1), (1, 2), (1, 3), (2, 0),
                  (2, 1), (2, 2), (2, 3), (3, 0), (3, 1)]
        for (p, s_), (bqb, bt) in zip(qseq, blocks):
            for q in range(4):
                put(bqb, bt, 2 * q, QQ(p, s_, q))
        # deferred A-transposes: a pair's transposes run in the next block at
        # kt5, after the cross-block PV tail + normalize have drained
        atseq = [(qb, t) for qb in range(QB) for t in range(4)][:-1]
        atblk = [(qb, t) for qb in range(QB) for t in range(4)][1:]
        for (aqb, at_), (bqb, bt) in zip(atseq, atblk):
            put(bqb, bt, 3 if (bqb, bt) == (3, 3) else 5, AT(aqb, at_))
        # o-proj for qb leveled across the next q-range (qb 0..2 -> qb+1),
        # starting after that q-range's at tiles are complete

        ooslots = [(0, 7), (0, 9), (0, 11), (0, 13), (0, 15),
                   (1, 1), (1, 3), (1, 7), (1, 9), (1, 11), (1, 13),
                   (2, 1), (2, 3), (2, 7), (2, 9), (2, 11)]
        for qb in range(3):
            for ot, (bt, kt_) in enumerate(ooslots):
                put(qb + 1, bt, kt_, OO(qb, ot))
        # last q-range: pairs 0-2 of its o-proj during the final block
        o3kt = [4, 4, 5, 5, 6, 6, 7, 7, 8, 9, 10, 11, 12, 13, 14, 15]
        for ot in range(ST):
            put(3, 3, o3kt[ot], O3(ot))

        # ---- pre-attention: stripe-0 K and pair-0 Q (V(0) rides the first
        # two chunks) ----
        emit_qpad_alloc(0)
        emit_k(0, 0)
        emit_k(0, 1)
        emit_q(0, 0, 0)
        emit_q(0, 0, 1)
        emit_q(0, 0, 2)
        emit_q(0, 0, 3)

        # ---- attention: qb outer, head-pair inner; PV trails scores by two
        # chunks. The last chunk's exp is split into two 512-wide halves so
        # the trailing PV group waits on half an exp, not a whole one.
        for qb in range(QB):
            qcols = slice(qb * 512, (qb + 1) * 512)
            a_tiles = [ap_.tile([128, QCH], BF16, tag="a", name=f"a{qb}_{i}")
                       for i in range(4)]
            a_of[qb] = a_tiles
            for t in range(4):
                qpad = qpad_of[t]
                ptile = [None] * ST
                pa = [pap.tile([128, 260], F32, tag="pa", name=f"pa{h}")
                      for h in range(2)]

                def emit_pv(kt, pa=pa, ptile=ptile):
                    for h in range(2):
                        for qt in range(4):
                            nc.tensor.matmul(
                                pa[h][:, qt * 65:qt * 65 + 65],
                                ptile[kt][:, h * 512 + qt * 128:
                                          h * 512 + (qt + 1) * 128],
                                vones[kt][:, h * 65:h * 65 + 65],
                                start=(kt == 0 and qt == 0),
                                stop=(kt == ST - 1 and qt == 3),
                                skip_group_check=True)

                def norm(pa=pa, a_tiles=a_tiles, t=t):
                    for h in range(2):
                        slot = 2 * t + h
                        for qt in range(4):
                            rc = rcp.tile([128, 1], F32, tag="rc")
                            nc.vector.reciprocal(
                                rc[:], pa[h][:, qt * 65 + 64:qt * 65 + 65])
                            nc.vector.tensor_scalar_mul(
                                a_tiles[qt][:, slot * 64:(slot + 1) * 64],
                                pa[h][:, qt * 65:qt * 65 + 64], rc[:])

                for kt in range(ST):
                    ps2 = big.tile([128, 1024], F32, tag="big")
                    for h in range(2):
                        nc.tensor.matmul(
                            ps2[:, h * 512:(h + 1) * 512],
                            kt_sb[:, kt * 128:(kt + 1) * 128],
                            qpad[h][:, qcols],
                            start=True, stop=True)
                    pe = pexp.tile([128, 1024], BF16, tag="pexp")
                    if kt == ST - 1:
                        for h in range(2):
                            nc.scalar.activation(
                                pe[:, h * 512:(h + 1) * 512],
                                ps2[:, h * 512:(h + 1) * 512],
                                mybir.ActivationFunctionType.Exp, scale=0.125)
                    else:
                        nc.scalar.activation(pe[:], ps2[:],
                                             mybir.ActivationFunctionType.Exp,
                                             scale=0.125)
                    ptile[kt] = pe
                    if kt >= 2:
                        emit_pv(kt - 2)
                    for f in sched.get((qb, t, kt), []):
                        f()
                emit_pv(ST - 2)
                emit_pv(ST - 1)
                norm()

        # tail: pair 3 of the last q-range
        emit_at(QB - 1, 3)
        for ot in range(ST):
            emit_oproj3_tail(ot)

    nc.compile()
    return nc


def _get_nc():
    global _CACHED_NC
    if _CACHED_NC is None:
        _CACHED_NC = _build_nc()
    return _CACHED_NC


def _arr4(a2d):
    """[H, n] -> [128, HK, n] hk-stripe layout, bf16 contiguous."""
    n = a2d.shape[1]
    return np.ascontiguousarray(
        a2d.reshape(HK, 128, n).transpose(1, 0, 2)).astype(BF16_NP)


def _arr5(xT):
    """[H, S] -> [NS, 128, HK, 512] stripe-major layout, bf16 contiguous."""
    return np.ascontiguousarray(
        xT.reshape(HK, 128, NS, 512).transpose(2, 1, 0, 3)).astype(BF16_NP)


def _prep_core_inputs(hidden_states, Wq, Wk, Wv, Wo):
    """Host-side shard + transpose + bf16 cast. Returns list of 8 input dicts."""
    xT_b = []
    for b in range(B):
        xT_b.append(_arr5(np.ascontiguousarray(hidden_states[b].T)))
    in_maps = []
    for c in range(N_CORES):
        b, g = divmod(c, TP)
        wq_rows = np.concatenate([
            Wq[(g * QH + h) * HEAD_DIM:(g * QH + h + 1) * HEAD_DIM, :]
            for h in HEAD_ORDER], axis=0)            # [512, H]
        wo_cols = np.concatenate([
            Wo[:, (g * QH + h) * HEAD_DIM:(g * QH + h + 1) * HEAD_DIM]
            for h in HEAD_ORDER], axis=1)            # [H, 512]
        wq4 = _arr4(np.ascontiguousarray(wq_rows.T))  # [128, HK, 512]
        wq5 = np.ascontiguousarray(wq4.reshape(128, HK, 4, 128)
                                   .transpose(2, 0, 1, 3))
        in_maps.append({
            "xT5": xT_b[b],
            "wq5": wq5,
            "wk4": _arr4(np.ascontiguousarray(Wk[g * KCH:(g + 1) * KCH, :].T)),
            "wv4": _arr4(np.ascontiguousarray(Wv[g * KCH:(g + 1) * KCH, :].T)),
            "woT": np.ascontiguousarray(wo_cols.T).astype(BF16_NP),
        })
    return in_maps


def _combine(results):
    out = np.empty((B, S, H), dtype=np.float32)
    for b in range(B):
        acc = np.concatenate(
            [results[b * TP]["outT"].astype(np.float32),
             results[b * TP]["outT3"]], axis=1)
        for g in range(1, TP):
            r = results[b * TP + g]
            acc[:, 0:1536] += r["outT"].astype(np.float32)
            acc[:, 1536:2048] += r["outT3"]
        out[b] = acc.T
    return out


def kernel(hidden_states, attention_mask, Wq, Wk, Wv, Wo):
    # attention_mask is all zeros for this problem spec; softmax is invariant
    # to the zero additive mask, so it is not shipped to the device.
    hidden_states = np.asarray(hidden_states)
    nc = _get_nc()
    in_maps = _prep_core_inputs(hidden_states, np.asarray(Wq), np.asarray(Wk),
                                np.asarray(Wv), np.asarray(Wo))
    res = run_bass_kernel_spmd(nc, in_maps, list(range(N_CORES)))
    return _combine(res.results)
